# revision 28
# baseline (speedup 1.0000x reference)
"""Low-rank (LoRA) linear for Trainium2, 8 NeuronCores.

Reference math:  out = x @ W^T + b + (ALPHA/R) * (x @ A^T) @ B^T
  x: (4, 2048, 4096) f32, W: (4096, 4096), b: (4096,), A: (16, 4096), B: (4096, 16)

Strategy (v2):
  * Fold the adapter on the host: W_eff = W + SCALE * (B @ A); the kernel is a
    single dense GEMM  out = x @ W_eff^T + b.
  * Data-parallel over tokens: 8192 tokens -> 8 cores x 1024 tokens.
  * lhsT = x^T tile (bf16, stationary), rhs = W_eff^T (fp8 e3m4 x128, moving).
    One-sided e3m4 keeps rel err ~1.15e-2 (< 2e-2 gate) while halving W DMA
    bytes; fp8 streams at bf16 speed (1 col/cycle), so compute is unchanged:
    per core M=1024,K=4096,N=4096 -> 34.4 GFLOP, PE roofline ~437 us.
  * Startup is DMA-bound (~330 GB/s aggregate over 3 rings, ~6.5 us engine
    preamble).  All transfers are issued in global deadline order, split
    across the sync/scalar(act)/gpsimd rings: bias row, W-block-0 + x-tile-0
    in 256 KB chunks (PE trickles matmuls as chunks land), then x1..x7 split
    3-way, then W blocks 1..7 split 2-way (sync+gpsimd).  Outputs get the
    scalar ring to themselves to avoid head-of-line blocking.
  * A few zero matmuls at t~6.3us keep the PE busy so the HAM clock gate
    reaches 8/8 (~2.4 GHz) before the real stream begins.
  * bias is shipped as a [1,4096] row and broadcast to [128,4096] on-device
    with K=1 ones-matmuls; eviction fuses descale (1/128) + bias add in one
    DVE scalar_tensor_tensor, writing bf16 (upcast to f32 on host).
"""

import os

os.environ.setdefault("MYCRO_LOCAL_CACHE", "1")

import numpy as np
import ml_dtypes

R = 16
ALPHA = 32.0
SCALE = ALPHA / R

P = 128          # partitions
D = 4096         # d_in (contraction)
O = 4096         # d_out
S_FULL = 8192    # 4*2048 tokens
N_CORES = 8
S = S_FULL // N_CORES   # tokens per core
DO = D // P             # 32 contraction chunks
ST = S // P             # 8 token tiles per core
NB = 512                # output cols per matmul (one PSUM bank, f32)
OE = O // NB            # 8 output-column blocks

W_SCALE = 128.0          # host-side premultiplier before e3m4 cast (W)
X_SCALE = 2.0            # premultiplier for the e3m4 x tiles (st 2..7)
DESCALE = 1.0 / W_SCALE
N_DUMMY = 26             # HAM warm-up matmuls on zeroed tiles

BF16 = ml_dtypes.bfloat16
E3M4 = ml_dtypes.float8_e3m4

_cache = {}


def _build_module():
    import concourse.mybir as mybir
    import concourse.tile as tile
    from concourse import bacc

    nc = bacc.Bacc(
        "TRN2", target_bir_lowering=False, debug=False, num_devices=N_CORES
    )
    xT = nc.dram_tensor(
        "xT", (2, P, DO, P), mybir.dt.bfloat16, kind="ExternalInput"
    ).ap()
    xTq = nc.dram_tensor(
        "xTq", (ST - 2, P, DO, P), mybir.dt.float8e3, kind="ExternalInput"
    ).ap()
    wT = nc.dram_tensor(
        "wT", (OE, P, DO, NB), mybir.dt.float8e3, kind="ExternalInput"
    ).ap()
    bv = nc.dram_tensor("bv", (1, O), mybir.dt.bfloat16, kind="ExternalInput").ap()
    on = nc.dram_tensor("on", (1, P), mybir.dt.bfloat16, kind="ExternalInput").ap()
    out = nc.dram_tensor("out", (S, O), mybir.dt.bfloat16, kind="ExternalOutput").ap()

    XC = 4            # x-tile-0 startup chunks (8 do each)
    W0C = 8           # W-block-0 startup chunks (4 do each)
    mult = mybir.AluOpType.mult
    add = mybir.AluOpType.add

    with tile.TileContext(nc) as tc:
        with tc.tile_pool(name="xp", bufs=1) as xp, \
             tc.tile_pool(name="w0p", bufs=1) as w0p, \
             tc.tile_pool(name="wp", bufs=3) as wp, \
             tc.tile_pool(name="bp", bufs=1) as bp, \
             tc.tile_pool(name="dp", bufs=1) as dp, \
             tc.tile_pool(name="op", bufs=8) as op, \
             tc.tile_pool(name="pp", bufs=4, space="PSUM") as pp, \
             tc.tile_pool(name="ppb", bufs=2, space="PSUM") as ppb, \
             tc.tile_pool(name="ppd", bufs=1, space="PSUM") as ppd:

            # ---- tiles -------------------------------------------------
            x0c = [xp.tile([P, 8, P], mybir.dt.bfloat16, tag=f"x0c{j}", name=f"x0c{j}")
                   for j in range(XC)]
            x_t = {st: xp.tile(
                       [P, DO, P],
                       mybir.dt.bfloat16 if st == 1 else mybir.dt.float8e3,
                       tag=f"x{st}", name=f"x{st}")
                   for st in range(1, ST)}
            w0c = [w0p.tile([P, 4, NB], mybir.dt.float8e3, tag=f"w0c{j}", name=f"w0c{j}")
                   for j in range(W0C)]
            w_t = {b: wp.tile([P, DO, NB], mybir.dt.float8e3, tag="w", name=f"wt{b}")
                   for b in range(1, OE)}
            bvec_sb = bp.tile([1, O], mybir.dt.bfloat16)
            ones_sb = bp.tile([1, P], mybir.dt.bfloat16)
            bias_sb = bp.tile([P, O], mybir.dt.float32)
            dum_l = dp.tile([P, P], mybir.dt.bfloat16)
            dum_r = dp.tile([P, NB], mybir.dt.bfloat16)

            # ---- warm-up: PE busy from the end of the preamble ---------
            nc.vector.memset(dum_l[:], 0.0)
            nc.vector.memset(dum_r[:], 0.0)
            psd = ppd.tile([P, NB], mybir.dt.float32)
            for _ in range(N_DUMMY):
                nc.tensor.matmul(psd[:], dum_l[:], dum_r[:], start=True, stop=True)

            # ---- DMA ring programs (per-engine FIFO = priority order) --
            # The gpsimd (software-DGE) ring starts ~5us after the two
            # HWDGE rings, so it carries only the later-deadline items:
            # x-tile-0 chunks (consumed do-group by do-group) and tail
            # thirds.  W block 0 goes on the HW rings, bias row first on
            # scalar (needed by the PE bias broadcast at ~13us).
            nc.scalar.dma_start(out=bvec_sb[:], in_=bv[:])
            nc.scalar.dma_start(out=ones_sb[:], in_=on[:])
            for j in range(0, W0C, 2):
                nc.sync.dma_start(out=w0c[j][:], in_=wT[0, :, 4 * j:4 * j + 4, :])
            for j in range(1, W0C, 2):
                nc.scalar.dma_start(out=w0c[j][:], in_=wT[0, :, 4 * j:4 * j + 4, :])
            for j in range(XC):
                nc.gpsimd.dma_start(
                    out=x0c[j][:], in_=xT[0, :, 8 * j:8 * j + 8, :]
                )
            # x tile 1 right after W0 on the HW rings (st1 deadline);
            # x tiles 2..7 split 12/12/8 with gpsimd taking the tail.
            nc.sync.dma_start(out=x_t[1][:, 0:16, :], in_=xT[1, :, 0:16, :])
            nc.scalar.dma_start(out=x_t[1][:, 16:32, :], in_=xT[1, :, 16:32, :])
            for st in range(2, ST):
                nc.sync.dma_start(out=x_t[st][:, 0:12, :], in_=xTq[st - 2, :, 0:12, :])
                nc.scalar.dma_start(out=x_t[st][:, 12:24, :], in_=xTq[st - 2, :, 12:24, :])
                nc.gpsimd.dma_start(out=x_t[st][:, 24:32, :], in_=xTq[st - 2, :, 24:32, :])
            # W blocks 1..7: halves on sync+gpsimd (wp bufs=3 throttles)
            for b in range(1, OE):
                nc.sync.dma_start(out=w_t[b][:, 0:16, :], in_=wT[b, :, 0:16, :])
                nc.gpsimd.dma_start(out=w_t[b][:, 16:32, :], in_=wT[b, :, 16:32, :])

            # ---- bias broadcast: [1,O] -> [128,O] via K=1 matmuls ------
            for j in range(OE):
                psb = ppb.tile([P, NB], mybir.dt.float32, tag="pb")
                nc.tensor.matmul(
                    psb[:], ones_sb[:], bvec_sb[:, j * NB:(j + 1) * NB],
                    start=True, stop=True,
                )
                nc.vector.tensor_copy(bias_sb[:, j * NB:(j + 1) * NB], psb[:])

            # ---- main GEMM ---------------------------------------------
            # short dummy chains bridge the x3/x4 arrival gaps in block 0
            # so the free-running HAM MID window never re-throttles the PE
            FILLERS = {3: 4, 4: 8}
            for oe in range(OE):
                for st in range(ST):
                    if oe == 0 and st in FILLERS:
                        for _ in range(FILLERS[st]):
                            nc.tensor.matmul(
                                psd[:], dum_l[:], dum_r[:],
                                start=True, stop=True, skip_group_check=True,
                            )
                    ps = pp.tile([P, NB], mybir.dt.float32, tag="ps")
                    for do in range(DO):
                        if st == 0:
                            lhsT = x0c[do // 8][:, do % 8, :]
                        else:
                            lhsT = x_t[st][:, do, :]
                        if oe == 0:
                            rhs = w0c[do // 4][:, do % 4, :]
                        else:
                            rhs = w_t[oe][:, do, :]
                        nc.tensor.matmul(
                            ps[:], lhsT, rhs,
                            start=(do == 0), stop=(do == DO - 1),
                        )
                    o_sb = op.tile([P, NB], mybir.dt.bfloat16, tag="o")
                    orow = out[st * P:(st + 1) * P, oe * NB:(oe + 1) * NB]
                    bias = bias_sb[:, oe * NB:(oe + 1) * NB]
                    if oe == OE - 1 and st == ST - 1:
                        # split the last eviction so the tail drains fast
                        h = NB // 2
                        nc.vector.scalar_tensor_tensor(
                            o_sb[:, 0:h], ps[:, 0:h], DESCALE, bias[:, 0:h],
                            mult, add,
                        )
                        nc.scalar.dma_start(out=orow[:, 0:h], in_=o_sb[:, 0:h])
                        nc.vector.scalar_tensor_tensor(
                            o_sb[:, h:NB], ps[:, h:NB], DESCALE, bias[:, h:NB],
                            mult, add,
                        )
                        nc.sync.dma_start(out=orow[:, h:NB], in_=o_sb[:, h:NB])
                    else:
                        nc.vector.scalar_tensor_tensor(
                            o_sb[:], ps[:], DESCALE, bias, mult, add,
                        )
                        nc.scalar.dma_start(out=orow, in_=o_sb[:])
    nc.compile()
    return nc


def _get_module():
    if "nc" not in _cache:
        _cache["nc"] = _build_module()
    return _cache["nc"]


def _prep_inputs(x, W, b, A, B):
    """Host-side: fold adapter, transpose to kernel layouts, cast, shard."""
    W_eff = W.astype(np.float32) + SCALE * (
        B.astype(np.float32) @ A.astype(np.float32)
    )
    # wT[oe, p, do, oo] = W_eff[oe*NB+oo, do*P+p] * W_SCALE  (e3m4)
    wq = np.clip(W_eff * W_SCALE, -15.5, 15.5)
    wT = np.ascontiguousarray(
        wq.T.reshape(DO, P, OE, NB).transpose(2, 1, 0, 3)
    ).astype(E3M4)
    bvec = np.ascontiguousarray(b.astype(np.float32).reshape(1, O)).astype(BF16)
    ones = np.ones((1, P), dtype=BF16)
    x2 = np.asarray(x, dtype=np.float32).reshape(S_FULL, D)
    in_maps = []
    for c in range(N_CORES):
        xc = x2[c * S:(c + 1) * S]                       # (S, D)
        # xT[st, p, do, s'] = xc[st*P+s', do*P+p]  (contiguous per (st, p))
        xt = np.ascontiguousarray(
            xc.reshape(ST, P, DO, P).transpose(0, 3, 2, 1)
        )
        xTc = xt[:2].astype(BF16)
        # tokens of st 2..7 ride in e3m4 (x2 scale, descale folded below):
        # psum for those tiles is X_SCALE too large -> the eviction descale
        # must NOT differ per tile, so fold 1/X_SCALE into the tile itself
        # by pre-dividing?  No: scale then descale uniformly is impossible
        # per-tile; instead scale x and fold into W descale is wrong too.
        # Solution: quantize without upscaling but guard subnormals by the
        # X_SCALE pre-multiply and a matching 1/X_SCALE on the W side is
        # not needed: e3m4 handles |x|~1 fine (normals down to 0.25,
        # subnormal step 2^-6 = 0.0156 abs).  Direct cast measured
        # rel err 0.0115 per-tensor at s=1; acceptable here.
        xTq = xt[2:].astype(E3M4)
        in_maps.append(
            {"xT": xTc, "xTq": xTq, "wT": wT, "bv": bvec, "on": ones}
        )
    return in_maps


def run(x, W, b, A, B, trace=False, **spmd_kwargs):
    """Run the kernel; returns (full_output, BassKernelResults)."""
    from concourse import bass_utils

    nc = _get_module()
    in_maps = _prep_inputs(x, W, b, A, B)
    res = bass_utils.run_bass_kernel_spmd(
        nc, in_maps, core_ids=list(range(N_CORES)), trace=trace, **spmd_kwargs
    )
    outs = [
        np.asarray(res.results[c]["out"]).astype(np.float32)
        for c in range(N_CORES)
    ]
    full = np.concatenate(outs, axis=0).reshape(4, 2048, O)
    return full, res


def kernel(x, W, b, A, B):
    full, _ = run(x, W, b, A, B, trace=False)
    return full
